# revision 1
# baseline (speedup 1.0000x reference)
"""Causal self-attention (B=4, T=2048, D=1024, H=16) on 8 TRN2 NeuronCores.

Sharding: tensor-parallel over 4 head-groups x data-parallel over 2 batch-groups.
Core c handles batches [2*(c//4), 2*(c//4)+2) and heads [4*(c%4), 4*(c%4)+4).
Each core computes a partial output projection (its 256 feature rows of W_proj);
the host sums the 4 head-group partials per batch group.

All matmuls run in fp32r (fp32 with 11-bit mantissa, full PE rate for free>=256);
accumulation is fp32 in PSUM. x and the weight slices are RNE-rounded to fp32r
on the host. Softmax skips max-subtraction (scores are ~N(0,1), bounded well
inside fp32 exp range) so softmax(s) = exp(s)/sum(exp(s)) exactly.

Perf notes (from HW traces): the PE only reaches its warm clock on sustained
runs of identical matmul shapes; mixed shapes throttle it to half rate. So the
K and V weight slices are zero-padded on the host so that S^T (= K^T_pad.T @
Q^T, contraction 128 with zeros over the co-packed head) and PV (= V_pad.T @ P,
65 live output rows of 128) use the same [128,128,N] shape as the projections.
Attention issues all S^T matmuls of a (head, q-block) first, then all PV
matmuls, to maximize same-shape run length. The causal diagonal is handled by
slicing S^T/exp/PV to the valid q-range plus one [128,128] triangle mask mul.
"""
import functools
from contextlib import ExitStack

import numpy as np

import concourse.bacc as bacc
import concourse.tile as tile
import concourse.mybir as mybir
from concourse.bass_utils import run_bass_kernel_spmd
from concourse.masks import make_upper_triangular

F32 = mybir.dt.float32
F32R = mybir.dt.float32r
EXP = mybir.ActivationFunctionType.Exp

B, T, D, H, HD = 4, 2048, 1024, 16, 64
NB, NH = 2, 4            # batches / heads per core
DL = NH * HD             # local feature dim (256)
NC = 8
WCOL = 768               # per-dk weight columns: Q(256) K(256) V(256) packed


def round_f32r(a: np.ndarray) -> np.ndarray:
    """RNE-round fp32 to fp32r (keep top 20 bits: 1s+8e+11m). Matches HW."""
    u = np.ascontiguousarray(a).view(np.uint32).astype(np.uint64)
    lsb = (u >> 12) & 1
    u = (u + 0x7FF + lsb) & 0xFFFFF000
    return u.astype(np.uint32).view(np.float32).reshape(a.shape)


@functools.lru_cache(maxsize=1)
def build():
    nc = bacc.Bacc("TRN2", target_bir_lowering=False, debug=False, num_devices=NC)
    x_d = nc.dram_tensor("x", [NB, T, D], F32R, kind="ExternalInput").ap()
    wqkv_d = nc.dram_tensor("wqkv", [D, WCOL], F32R, kind="ExternalInput").ap()
    wproj_d = nc.dram_tensor("wproj", [DL, D], F32R, kind="ExternalInput").ap()
    ones_d = nc.dram_tensor("ones64", [128, 64], F32R, kind="ExternalInput").ap()
    ident_d = nc.dram_tensor("ident", [128, 128], F32R, kind="ExternalInput").ap()
    out_d = nc.dram_tensor("out", [NB, T, D], F32, kind="ExternalOutput").ap()

    NT5 = T // 512           # 4  (512-token super chunks)
    NTT = T // 128           # 16 (128-token chunks)
    NDK = D // 128           # 8  (feature chunks of input dim)

    with tile.TileContext(nc) as tc, ExitStack() as ctx:
        const = ctx.enter_context(tc.tile_pool(name="const", bufs=1))
        wpool = ctx.enter_context(tc.tile_pool(name="w", bufs=1))

        ident = const.tile([128, 128], F32R)
        nc.sync.dma_start(ident[:], ident_d)
        ones64 = const.tile([128, 64], F32R)
        nc.sync.dma_start(ones64[:], ones_d)
        tri = const.tile([128, 128], F32)   # tri[k,q] = 1.0 iff q >= k
        make_upper_triangular(nc, tri[:], val=1.0, diag=True)
        ones_col = const.tile([128, 64], F32)
        nc.gpsimd.memset(ones_col[:], 1.0)
        zcf = const.tile([128, 1024], F32)
        nc.gpsimd.memset(zcf[:], 0.0)

        # weights: w_sb[:, dk*WCOL + c] = wqkv[dk*128 + p, c]
        w_sb = wpool.tile([128, NDK * WCOL], F32R)
        nc.sync.dma_start(
            w_sb[:].rearrange("p (a c) -> p a c", a=NDK),
            wqkv_d.rearrange("(a p) c -> p a c", p=128))
        wp_sb = wpool.tile([128, 2 * D], F32R)
        nc.sync.dma_start(
            wp_sb[:].rearrange("p (a c) -> p a c", a=2),
            wproj_d.rearrange("(a p) c -> p a c", p=128))

        xin_pool = ctx.enter_context(tc.tile_pool(name="xin", bufs=3))
        for b in range(NB):
            with tc.tile_pool(name="actv", bufs=1) as actv:
                # Q^T packed: 2 chunks of 128 rows (2 heads each)
                qt = [actv.tile([128, T], F32R, tag=f"qt{cc}", name=f"qt{cc}")
                      for cc in range(2)]
                # K^T per head, rows 64*(h%2)..+64 live, rest zero (from matmul)
                kt = [actv.tile([128, T], F32R, tag=f"kt{h}", name=f"kt{h}")
                      for h in range(NH)]
                # V blocks per (token-tile ti, head h): 128 cols at (ti*4+h)*128:
                # cols 0-63 = V, col 64 = ones, 65-127 = zero (from matmul)
                v_sb = actv.tile([128, NTT * NH * 128], F32R, tag="v")
                # dead halves of per-head K^T tiles are zero
                for h in range(NH):
                    dead = slice(64, 128) if h % 2 == 0 else slice(0, 64)
                    for q2 in range(2):
                        nc.vector.tensor_copy(
                            kt[h][dead, 1024 * q2:1024 * q2 + 1024], zcf[dead, :])

                # ---- Phase A: x^T (PE transpose), Q^T, K^T, V ----
                with tc.tile_pool(name="xt", bufs=2) as xt_pool, \
                     tc.tile_pool(name="psT", bufs=3, space="PSUM") as psT, \
                     tc.tile_pool(name="psQK", bufs=2, space="PSUM") as psQK, \
                     tc.tile_pool(name="psV", bufs=2, space="PSUM") as psV:
                    for t5 in range(NT5):
                        xas = []
                        for half in range(2):
                            xa = xin_pool.tile([128, 2 * D], F32R, tag="xa",
                                               name=f"xa{half}")
                            nc.scalar.dma_start(
                                xa[:].rearrange("p (a c) -> p a c", a=2),
                                x_d[b, 512 * t5 + 256 * half:512 * t5 + 256 * half + 256]
                                .rearrange("(a p) c -> p a c", p=128))
                            xas.append(xa)
                        xt = [xt_pool.tile([128, 512], F32R, tag=f"xt{dk}",
                                           name=f"xt{dk}") for dk in range(NDK)]
                        # x^T: 2 transposes into one PSUM tile, 1 evac each
                        for half in range(2):
                            for dk in range(NDK):
                                pt = psT.tile([128, 256], F32R, tag="pt")
                                for tt in range(2):
                                    nc.tensor.matmul(
                                        pt[:, tt * 128:tt * 128 + 128],
                                        xas[half][:, tt * D + dk * 128:tt * D + dk * 128 + 128],
                                        ident[:], is_transpose=True,
                                        start=(tt == 0), stop=(tt == 1))
                                nc.scalar.copy(
                                    xt[dk][:, 256 * half:256 * half + 256], pt[:])
                        for cc in range(2):     # Q^T
                            ps = psQK.tile([128, 512], F32, tag="qk")
                            for dk in range(NDK):
                                nc.tensor.matmul(
                                    ps[:],
                                    w_sb[:, dk * WCOL + cc * 128:dk * WCOL + cc * 128 + 128],
                                    xt[dk][:],
                                    start=(dk == 0), stop=(dk == NDK - 1))
                            nc.vector.tensor_copy(
                                qt[cc][:, t5 * 512:t5 * 512 + 512], ps[:])
                        for cc in range(2):     # K^T packed; split to heads
                            ps = psQK.tile([128, 512], F32, tag="qk")
                            for dk in range(NDK):
                                nc.tensor.matmul(
                                    ps[:],
                                    w_sb[:, dk * WCOL + 256 + cc * 128:dk * WCOL + 256 + cc * 128 + 128],
                                    xt[dk][:],
                                    start=(dk == 0), stop=(dk == NDK - 1))
                            nc.vector.tensor_copy(
                                kt[2 * cc][0:64, t5 * 512:t5 * 512 + 512],
                                ps[0:64, :])
                            nc.vector.tensor_copy(
                                kt[2 * cc + 1][64:128, t5 * 512:t5 * 512 + 512],
                                ps[64:128, :])
                        v128 = v_sb[:].rearrange("p (n c) -> p n c", c=128)
                        for tt in range(4):     # V packed (N=256)
                            ps = psV.tile([128, 256], F32, tag="v")
                            for dk in range(NDK):
                                nc.tensor.matmul(
                                    ps[:],
                                    xt[dk][:, tt * 128:tt * 128 + 128],
                                    w_sb[:, dk * WCOL + 512:dk * WCOL + 768],
                                    start=(dk == 0), stop=(dk == NDK - 1))
                            ti = t5 * 4 + tt
                            nc.vector.tensor_copy(
                                v128[:, ti * 4:ti * 4 + 4, 0:64],
                                ps[:].rearrange("p (n c) -> p n c", c=64))
                        # ones col + zero cols of each 128-block of this chunk
                        nc.vector.tensor_copy(
                            v128[:, 16 * t5:16 * (t5 + 1), 64],
                            ones_col[:, 0:16])
                        nc.vector.tensor_copy(
                            v128[:, 16 * t5:16 * (t5 + 1), 65:128],
                            zcf[:, 0:1008].rearrange("p (n c) -> p n c", c=63))

                # ---- Phase B: attention ----
                # All S^T matmuls of a (h,j) first (exps stream on ACT), then
                # all PV matmuls: long same-shape runs keep the PE at full clock.
                with tc.tile_pool(name="psS", bufs=2, space="PSUM") as psS_pool, \
                     tc.tile_pool(name="psY", bufs=1, space="PSUM") as psY_pool, \
                     tc.tile_pool(name="psBC", bufs=1, space="PSUM") as psBC, \
                     tc.tile_pool(name="psO", bufs=2, space="PSUM") as psO_pool, \
                     tc.tile_pool(name="pP", bufs=9) as pP, \
                     tc.tile_pool(name="ytp", bufs=2) as ytp, \
                     tc.tile_pool(name="ost", bufs=2) as ost_pool, \
                     tc.tile_pool(name="ysm", bufs=2) as ysm:
                    for j in range(NT5):
                        yt = [ytp.tile([128, 512], F32R, tag=f"yt{ff}",
                                       name=f"yt{ff}") for ff in range(2)]
                        for h in range(NH):
                            ro = 64 * (h % 2)
                            qth = qt[h // 2]
                            nk = 4 * j + 4
                            offs = [128 * (i - 4 * j) if i - 4 * j > 0 else 0
                                    for i in range(nk)]
                            Ps = []
                            for m in range(nk // 2):
                                psS = psS_pool.tile([128, 1024], F32, tag="s",
                                                    name=f"psS{m}")
                                P = pP.tile([128, 1024], F32R, tag="p",
                                            name=f"P{m}")
                                Ps.append(P)
                                for c in (0, 1):
                                    i = 2 * m + c
                                    off = offs[i]
                                    nc.tensor.matmul(
                                        psS[:, c * 512 + off:(c + 1) * 512],
                                        kt[h][:, 128 * i:128 * i + 128],
                                        qth[:, 512 * j + off:512 * (j + 1)],
                                        start=True, stop=True)
                                if 2 * m + 1 < 4 * j or 2 * m == 4 * j:
                                    # t0/t1 diagonal pair: exp the whole tile in
                                    # one op; cols 512..640 are never read by PV
                                    nc.scalar.activation(P[:], psS[:], EXP, scale=0.125)
                                else:
                                    for c in (0, 1):
                                        off = offs[2 * m + c]
                                        nc.scalar.activation(
                                            P[:, c * 512 + off:(c + 1) * 512],
                                            psS[:, c * 512 + off:(c + 1) * 512],
                                            EXP, scale=0.125)
                                for c in (0, 1):
                                    i = 2 * m + c
                                    if i >= 4 * j:
                                        off = offs[i]
                                        nc.vector.tensor_mul(
                                            P[:, c * 512 + off:c * 512 + off + 128],
                                            P[:, c * 512 + off:c * 512 + off + 128],
                                            tri[:].bitcast(F32R))
                            psY = psY_pool.tile([128, 512], F32, tag="y")
                            for i in range(nk):
                                off = offs[i]
                                nc.tensor.matmul(
                                    psY[:, off:512],
                                    v_sb[:, 512 * i + 128 * h:512 * i + 128 * h + 128],
                                    Ps[i // 2][:, (i % 2) * 512 + off:(i % 2 + 1) * 512],
                                    start=(i == 0), stop=(i == nk - 1))
                            # divide by the accumulated denominator (row 64)
                            ya = ysm.tile([65, 512], F32R, tag="ya")
                            nc.vector.tensor_copy(ya[:], psY[0:65, :])
                            bc = psBC.tile([64, 512], F32, tag="bc")
                            nc.tensor.matmul(bc[:], ones64[64:65, :], ya[64:65, :],
                                             start=True, stop=True)
                            rb = ysm.tile([64, 512], F32, tag="rb", bufs=1)
                            nc.vector.reciprocal_approx_fast(rb[:], bc[:])
                            nc.vector.tensor_mul(
                                yt[h // 2][ro:ro + 64, :],
                                ya[0:64, :], rb[:])
                        # ---- output projection for this 512-token block ----
                        for g2 in range(2):
                            ostage = ost_pool.tile([128, 2 * D], F32, tag="o")
                            for a in range(2):
                                tt = 2 * g2 + a
                                for nn2 in range(2):
                                    ps = psO_pool.tile([128, 512], F32, tag="o")
                                    for ff in range(2):
                                        nc.tensor.matmul(
                                            ps[:],
                                            yt[ff][:, 128 * tt:128 * tt + 128],
                                            wp_sb[:, ff * D + 512 * nn2:ff * D + 512 * nn2 + 512],
                                            start=(ff == 0), stop=(ff == 1))
                                    nc.vector.tensor_copy(
                                        ostage[:, a * D + 512 * nn2:a * D + 512 * nn2 + 512],
                                        ps[:])
                            nc.sync.dma_start(
                                out_d[b, 512 * j + 256 * g2:512 * j + 256 * g2 + 256]
                                .rearrange("(a p) c -> p a c", p=128),
                                ostage[:].rearrange("p (a c) -> p a c", a=2))

    nc.compile()
    return nc


def make_in_maps(x, W_qkv, W_proj):
    ones = np.ones((128, 64), dtype=np.float32)
    ident = np.eye(128, dtype=np.float32)
    in_maps = []
    for c in range(NC):
        bg, hg = c // 4, c % 4
        wq = np.concatenate(
            [W_qkv[:, 256 * hg:256 * hg + 256],
             W_qkv[:, 1024 + 256 * hg:1024 + 256 * hg + 256],
             W_qkv[:, 2048 + 256 * hg:2048 + 256 * hg + 256]], axis=1)
        in_maps.append({
            "x": round_f32r(np.ascontiguousarray(x[2 * bg:2 * bg + 2])),
            "wqkv": round_f32r(wq),
            "wproj": round_f32r(W_proj[256 * hg:256 * hg + 256, :]),
            "ones64": ones,
            "ident": ident,
        })
    return in_maps


def kernel(x, W_qkv, W_proj):
    x = np.asarray(x, dtype=np.float32)
    W_qkv = np.asarray(W_qkv, dtype=np.float32)
    W_proj = np.asarray(W_proj, dtype=np.float32)
    nc = build()
    res = run_bass_kernel_spmd(nc, make_in_maps(x, W_qkv, W_proj), list(range(NC)))
    out = np.zeros((B, T, D), dtype=np.float64)
    for c in range(NC):
        bg = c // 4
        out[2 * bg:2 * bg + 2] += res.results[c]["out"].astype(np.float64)
    return out.astype(np.float32)



# revision 17
# speedup vs baseline: 1.0154x; 1.0154x over previous
"""Causal self-attention (B=4, T=2048, D=1024, H=16) on 8 TRN2 NeuronCores.

Sharding: tensor-parallel over 4 head-groups x data-parallel over 2 batch-groups.
Core c handles batches [2*(c//4), 2*(c//4)+2) and heads [4*(c%4), 4*(c%4)+4).
Each core computes a partial output projection (its 256 feature rows of W_proj);
the host sums the 4 head-group partials per batch group.

All matmuls run in fp32r (fp32 with 11-bit mantissa, full PE rate for free>=256);
accumulation is fp32 in PSUM. x and the weight slices are RNE-rounded to fp32r
on the host. Softmax skips max-subtraction (scores are ~N(0,1), bounded well
inside fp32 exp range) so softmax(s) = exp(s)/sum(exp(s)) exactly.

Perf notes (from HW traces): the PE only reaches its warm clock on sustained
runs of identical matmul shapes; mixed shapes throttle it to half rate. So the
K and V weight slices are zero-padded on the host so that S^T (= K^T_pad.T @
Q^T, contraction 128 with zeros over the co-packed head) and PV (= V_pad.T @ P,
65 live output rows of 128) use the same [128,128,N] shape as the projections.
Attention issues all S^T matmuls of a (head, q-block) first, then all PV
matmuls, to maximize same-shape run length. The causal diagonal is handled by
slicing S^T/exp/PV to the valid q-range plus one [128,128] triangle mask mul.
"""
import functools
from contextlib import ExitStack

import numpy as np

import concourse.bacc as bacc
import concourse.tile as tile
import concourse.mybir as mybir
from concourse.bass_utils import run_bass_kernel_spmd
from concourse.masks import make_upper_triangular

F32 = mybir.dt.float32
F32R = mybir.dt.float32r
EXP = mybir.ActivationFunctionType.Exp

B, T, D, H, HD = 4, 2048, 1024, 16, 64
NB, NH = 2, 4            # batches / heads per core
DL = NH * HD             # local feature dim (256)
NC = 8
WCOL = 768               # per-dk weight columns: Q(256) K(256) V(256) packed


def round_f32r(a: np.ndarray) -> np.ndarray:
    """RNE-round fp32 to fp32r (keep top 20 bits: 1s+8e+11m). Matches HW."""
    u = np.ascontiguousarray(a).view(np.uint32).astype(np.uint64)
    lsb = (u >> 12) & 1
    u = (u + 0x7FF + lsb) & 0xFFFFF000
    return u.astype(np.uint32).view(np.float32).reshape(a.shape)


@functools.lru_cache(maxsize=1)
def build():
    nc = bacc.Bacc("TRN2", target_bir_lowering=False, debug=False, num_devices=NC)
    x_d = nc.dram_tensor("x", [NB, T, D], F32R, kind="ExternalInput").ap()
    wqkv_d = nc.dram_tensor("wqkv", [D, WCOL], F32R, kind="ExternalInput").ap()
    wproj_d = nc.dram_tensor("wproj", [DL, D], F32R, kind="ExternalInput").ap()
    ones_d = nc.dram_tensor("ones64", [128, 64], F32R, kind="ExternalInput").ap()
    ident_d = nc.dram_tensor("ident", [128, 128], F32R, kind="ExternalInput").ap()
    out_d = nc.dram_tensor("out", [NB, T, D], F32, kind="ExternalOutput").ap()

    NT5 = T // 512           # 4  (512-token super chunks)
    NTT = T // 128           # 16 (128-token chunks)
    NDK = D // 128           # 8  (feature chunks of input dim)

    with tile.TileContext(nc) as tc, ExitStack() as ctx:
        const = ctx.enter_context(tc.tile_pool(name="const", bufs=1))
        wpool = ctx.enter_context(tc.tile_pool(name="w", bufs=1))

        ident = const.tile([128, 128], F32R)
        nc.sync.dma_start(ident[:], ident_d)
        ones64 = const.tile([128, 64], F32R)
        nc.sync.dma_start(ones64[:], ones_d)
        tri = const.tile([128, 128], F32)   # tri[k,q] = 1.0 iff q >= k
        make_upper_triangular(nc, tri[:], val=1.0, diag=True)
        ones_col = const.tile([128, 64], F32)
        nc.gpsimd.memset(ones_col[:], 1.0)
        zcf = const.tile([128, 1024], F32)
        nc.gpsimd.memset(zcf[:], 0.0)

        # weights: w_sb[:, dk*WCOL + c] = wqkv[dk*128 + p, c]
        w_sb = wpool.tile([128, NDK * WCOL], F32R)
        nc.sync.dma_start(
            w_sb[:].rearrange("p (a c) -> p a c", a=NDK),
            wqkv_d.rearrange("(a p) c -> p a c", p=128))
        wp_sb = wpool.tile([128, 2 * D], F32R)
        nc.sync.dma_start(
            wp_sb[:].rearrange("p (a c) -> p a c", a=2),
            wproj_d.rearrange("(a p) c -> p a c", p=128))

        xin_pool = ctx.enter_context(tc.tile_pool(name="xin", bufs=3))
        for b in range(NB):
            with tc.tile_pool(name="actv", bufs=1) as actv:
                # Q^T packed: 2 chunks of 128 rows (2 heads each)
                qt = [actv.tile([128, T], F32R, tag=f"qt{cc}", name=f"qt{cc}")
                      for cc in range(2)]
                # K^T per head, rows 64*(h%2)..+64 live, rest zero (from matmul)
                kt = [actv.tile([128, T], F32R, tag=f"kt{h}", name=f"kt{h}")
                      for h in range(NH)]
                # V blocks per (token-tile ti, head h): 65 cols at (ti*4+h)*65:
                # cols 0-63 = V, col 64 = ones (accumulates the softmax denom)
                v_sb = actv.tile([128, NTT * NH * 65], F32R, tag="v")
                V65 = v_sb[:].rearrange("p (t h c) -> p t h c", h=NH, c=65)
                nc.vector.tensor_copy(
                    V65[:, :, :, 64],
                    ones_col[:, 0:64].rearrange("p (t h) -> p t h", h=NH))
                # dead halves of per-head K^T tiles are zero
                for h in range(NH):
                    dead = slice(64, 128) if h % 2 == 0 else slice(0, 64)
                    for q2 in range(2):
                        nc.vector.tensor_copy(
                            kt[h][dead, 1024 * q2:1024 * q2 + 1024], zcf[dead, :])

                # ---- Phase A: x^T (PE transpose), Q^T, K^T, V ----
                with tc.tile_pool(name="xt", bufs=2) as xt_pool, \
                     tc.tile_pool(name="psT", bufs=3, space="PSUM") as psT, \
                     tc.tile_pool(name="psQK", bufs=2, space="PSUM") as psQK, \
                     tc.tile_pool(name="psV", bufs=2, space="PSUM") as psV:
                    for t5 in range(NT5):
                        xas = []
                        for half in range(2):
                            xa = xin_pool.tile([128, 2 * D], F32R, tag="xa",
                                               name=f"xa{half}")
                            nc.scalar.dma_start(
                                xa[:].rearrange("p (a c) -> p a c", a=2),
                                x_d[b, 512 * t5 + 256 * half:512 * t5 + 256 * half + 256]
                                .rearrange("(a p) c -> p a c", p=128))
                            xas.append(xa)
                        xt = [xt_pool.tile([128, 512], F32R, tag=f"xt{dk}",
                                           name=f"xt{dk}") for dk in range(NDK)]
                        # x^T: 2 transposes into one PSUM tile, 1 evac each
                        for half in range(2):
                            for dk in range(NDK):
                                pt = psT.tile([128, 256], F32R, tag="pt")
                                for tt in range(2):
                                    nc.tensor.matmul(
                                        pt[:, tt * 128:tt * 128 + 128],
                                        xas[half][:, tt * D + dk * 128:tt * D + dk * 128 + 128],
                                        ident[:], is_transpose=True,
                                        start=(tt == 0), stop=(tt == 1))
                                nc.scalar.copy(
                                    xt[dk][:, 256 * half:256 * half + 256], pt[:])
                        for cc in range(2):     # Q^T
                            ps = psQK.tile([128, 512], F32, tag="qk")
                            for dk in range(NDK):
                                nc.tensor.matmul(
                                    ps[:],
                                    w_sb[:, dk * WCOL + cc * 128:dk * WCOL + cc * 128 + 128],
                                    xt[dk][:],
                                    start=(dk == 0), stop=(dk == NDK - 1))
                            nc.vector.tensor_copy(
                                qt[cc][:, t5 * 512:t5 * 512 + 512], ps[:])
                        for cc in range(2):     # K^T packed; split to heads
                            ps = psQK.tile([128, 512], F32, tag="qk")
                            for dk in range(NDK):
                                nc.tensor.matmul(
                                    ps[:],
                                    w_sb[:, dk * WCOL + 256 + cc * 128:dk * WCOL + 256 + cc * 128 + 128],
                                    xt[dk][:],
                                    start=(dk == 0), stop=(dk == NDK - 1))
                            nc.vector.tensor_copy(
                                kt[2 * cc][0:64, t5 * 512:t5 * 512 + 512],
                                ps[0:64, :])
                            nc.vector.tensor_copy(
                                kt[2 * cc + 1][64:128, t5 * 512:t5 * 512 + 512],
                                ps[64:128, :])
                        for tt in range(4):     # V packed (N=256)
                            ps = psV.tile([128, 256], F32, tag="v")
                            for dk in range(NDK):
                                nc.tensor.matmul(
                                    ps[:],
                                    xt[dk][:, tt * 128:tt * 128 + 128],
                                    w_sb[:, dk * WCOL + 512:dk * WCOL + 768],
                                    start=(dk == 0), stop=(dk == NDK - 1))
                            ti = t5 * 4 + tt
                            nc.vector.tensor_copy(
                                V65[:, ti, :, 0:64],
                                ps[:].rearrange("p (n c) -> p n c", c=64))

                # ---- Phase B: attention ----
                # All S^T matmuls of a (h,j) first (exps stream on ACT), then
                # all PV matmuls: long same-shape runs keep the PE at full clock.
                with tc.tile_pool(name="psS", bufs=2, space="PSUM") as psS_pool, \
                     tc.tile_pool(name="psY", bufs=2, space="PSUM") as psY_pool, \
                     tc.tile_pool(name="psO", bufs=2, space="PSUM") as psO_pool, \
                     tc.tile_pool(name="pP", bufs=9) as pP, \
                     tc.tile_pool(name="ytp", bufs=2) as ytp, \
                     tc.tile_pool(name="ost", bufs=2) as ost_pool, \
                     tc.tile_pool(name="ysm", bufs=2) as ysm:

                    def emit_proj(j, yt):
                        # ---- output projection for 512-token block j ----
                        for g2 in range(2):
                            ostage = ost_pool.tile([128, 2 * D], F32, tag="o")
                            for a in range(2):
                                tt = 2 * g2 + a
                                for nn2 in range(2):
                                    ps = psO_pool.tile([128, 512], F32, tag="o")
                                    for ff in range(2):
                                        nc.tensor.matmul(
                                            ps[:],
                                            yt[ff][:, 128 * tt:128 * tt + 128],
                                            wp_sb[:, ff * D + 512 * nn2:ff * D + 512 * nn2 + 512],
                                            start=(ff == 0), stop=(ff == 1))
                                    nc.vector.tensor_copy(
                                        ostage[:, a * D + 512 * nn2:a * D + 512 * nn2 + 512],
                                        ps[:])
                            nc.sync.dma_start(
                                out_d[b, 512 * j + 256 * g2:512 * j + 256 * g2 + 256]
                                .rearrange("(a p) c -> p a c", p=128),
                                ostage[:].rearrange("p (a c) -> p a c", a=2))

                    def emit_denom(psY, ya, yth, ro):
                        # broadcast Z (ya row 64) across 64 partitions via the
                        # PE (deferred past the next S group so the ya copy has
                        # drained); then yt = y * (1/Z)
                        bcp = psO_pool.tile([128, 512], F32, tag="o")
                        nc.tensor.matmul(bcp[0:64, :], ones64[64:65, :],
                                         ya[64:65, :], start=True, stop=True)
                        rb = ysm.tile([64, 512], F32, tag="rb")
                        nc.vector.reciprocal_approx_fast(rb[:], bcp[0:64, :])
                        nc.vector.tensor_mul(yth[ro:ro + 64, :], ya[0:64, :], rb[:])

                    pending = None
                    pending_bc = None
                    for j in range(NT5):
                        yt = [ytp.tile([128, 512], F32R, tag=f"yt{ff}",
                                       name=f"yt{ff}") for ff in range(2)]
                        for h in range(NH):
                            ro = 64 * (h % 2)
                            qth = qt[h // 2]
                            nk = 4 * j + 4
                            offs = [128 * (i - 4 * j) if i - 4 * j > 0 else 0
                                    for i in range(nk)]
                            Ps = []
                            for m in range(nk // 2):
                                psS = psS_pool.tile([128, 1024], F32, tag="s",
                                                    name=f"psS{m}")
                                P = pP.tile([128, 1024], F32R, tag="p",
                                            name=f"P{m}")
                                Ps.append(P)
                                for c in (0, 1):
                                    i = 2 * m + c
                                    off = offs[i]
                                    nc.tensor.matmul(
                                        psS[:, c * 512 + off:(c + 1) * 512],
                                        kt[h][:, 128 * i:128 * i + 128],
                                        qth[:, 512 * j + off:512 * (j + 1)],
                                        start=True, stop=True)
                                if 2 * m + 1 < 4 * j or 2 * m == 4 * j:
                                    # t0/t1 diagonal pair: exp the whole tile in
                                    # one op; cols 512..640 are never read by PV
                                    nc.scalar.activation(P[:], psS[:], EXP, scale=0.125)
                                else:
                                    for c in (0, 1):
                                        off = offs[2 * m + c]
                                        nc.scalar.activation(
                                            P[:, c * 512 + off:(c + 1) * 512],
                                            psS[:, c * 512 + off:(c + 1) * 512],
                                            EXP, scale=0.125)
                                for c in (0, 1):
                                    i = 2 * m + c
                                    if i >= 4 * j:
                                        off = offs[i]
                                        nc.vector.tensor_mul(
                                            P[:, c * 512 + off:c * 512 + off + 128],
                                            P[:, c * 512 + off:c * 512 + off + 128],
                                            tri[:].bitcast(F32R))
                            # previous block's projection goes here: its PE work
                            # overlaps this block's normalization chain
                            if h == 1 and pending is not None:
                                emit_proj(*pending)
                                pending = None
                            # deferred denominator of the PREVIOUS head: by now
                            # its ya copy has drained, so the bc matmul does not
                            # stall the PE
                            if pending_bc is not None:
                                emit_denom(*pending_bc)
                                pending_bc = None
                            psY = psY_pool.tile([128, 512], F32, tag="y")
                            for i in range(nk):
                                off = offs[i]
                                nc.tensor.matmul(
                                    psY[0:65, off:512],
                                    v_sb[:, 65 * (i * NH + h):65 * (i * NH + h) + 65],
                                    Ps[i // 2][:, (i % 2) * 512 + off:(i % 2 + 1) * 512],
                                    start=(i == 0), stop=(i == nk - 1))
                            ya = ysm.tile([65, 512], F32R, tag="ya")
                            nc.vector.tensor_copy(ya[:], psY[0:65, :])
                            pending_bc = (psY, ya, yt[h // 2], ro)
                        if pending_bc is not None:
                            emit_denom(*pending_bc)
                            pending_bc = None
                        pending = (j, yt)
                    if pending is not None:
                        emit_proj(*pending)
                        pending = None

    nc.compile()
    return nc


def make_in_maps(x, W_qkv, W_proj):
    ones = np.ones((128, 64), dtype=np.float32)
    ident = np.eye(128, dtype=np.float32)
    in_maps = []
    for c in range(NC):
        bg, hg = c // 4, c % 4
        wq = np.concatenate(
            [W_qkv[:, 256 * hg:256 * hg + 256],
             W_qkv[:, 1024 + 256 * hg:1024 + 256 * hg + 256],
             W_qkv[:, 2048 + 256 * hg:2048 + 256 * hg + 256]], axis=1)
        in_maps.append({
            "x": round_f32r(np.ascontiguousarray(x[2 * bg:2 * bg + 2])),
            "wqkv": round_f32r(wq),
            "wproj": round_f32r(W_proj[256 * hg:256 * hg + 256, :]),
            "ones64": ones,
            "ident": ident,
        })
    return in_maps


def kernel(x, W_qkv, W_proj):
    x = np.asarray(x, dtype=np.float32)
    W_qkv = np.asarray(W_qkv, dtype=np.float32)
    W_proj = np.asarray(W_proj, dtype=np.float32)
    nc = build()
    res = run_bass_kernel_spmd(nc, make_in_maps(x, W_qkv, W_proj), list(range(NC)))
    out = np.zeros((B, T, D), dtype=np.float64)
    for c in range(NC):
        bg = c // 4
        out[2 * bg:2 * bg + 2] += res.results[c]["out"].astype(np.float64)
    return out.astype(np.float32)

